# revision 16
# baseline (speedup 1.0000x reference)
"""GQA attention kernel for Trainium2 (8 NeuronCores, Bass/Tile) — v2.

Problem: B=2, S=2048, D=3072, 24 Q heads / 8 KV heads, HD=128, RoPE,
additive causal mask, softmax, output projection.

Sharding: tensor-parallel over heads. Core h owns KV head h and Q heads
{3h, 3h+1, 3h+2} for both batch elements. Each core produces a partial
y^T = wo_slice^T.T @ attn_out_heads^T of shape (B, D, S) in fp16; the
host sums the 8 partials in fp32 and transposes back.

v2 changes vs v1 (667us baseline):
  - Attention inner loop software-pipelined: scores(kt+1) is emitted
    before AV(kt)/rowsum(kt), so the in-order PE queue never waits on
    the ACT-engine exp.  (v1 lost ~500-1400ns per k-tile here.)
  - Out-projection of chunk qc-1 is deferred and drained between the
    attention heads of chunk qc as PE filler during ACT-paced stretches.
  - Causal diagonal blocks compute only the q >= r*128 live sub-range;
    the mask reduces to a single constant 128x128 upper-triangle tile
    multiplied into a 128-wide strip (no mask DMA traffic at all).
  - RoPE runs fully in fp16 (2x DVE rate), PSUM-drain copies are split
    across ACT and DVE, rotate-half stays a partition-shifted SBUF DMA.
  - V is transposed to [s, d] by the DMA crossbar (dma_start_transpose)
    instead of PE transposes + DVE copies.
  - yT output in fp16 (halves DRAM write traffic), cos/sin loaded once.
"""

import math
import os
import sys

import numpy as np

for _p in ("/opt/trn_rl_repo",):
    if os.path.isdir(_p) and _p not in sys.path:
        sys.path.insert(0, _p)

import concourse.bass as bass  # noqa: E402
import concourse.mybir as mybir  # noqa: E402
import concourse.tile as tile  # noqa: E402
from concourse import bacc  # noqa: E402
from concourse.bass_utils import run_bass_kernel_spmd  # noqa: E402

F32 = mybir.dt.float32
F32R = mybir.dt.float32r
F16 = mybir.dt.float16
AFT = mybir.ActivationFunctionType

N_CORES = 8

TRACE = False
LAST_EXEC_NS = None
LAST_RESULTS = None

B, S, D = 2, 2048, 3072
QH, HD, SC = 3, 128, 512
CT = D // 128          # 24 contraction tiles
KT = S // 128          # 16 key tiles
NSC = S // SC          # 4 token chunks
PB = SC // 128         # 4 key tiles per chunk
SCALE = 1.0 / math.sqrt(HD)

# Filler drain rate: out-projection mt-units emitted after each attention
# head (24 units per chunk / 3 heads).
FILL_PER_HEAD = 8


def build_program():
    nc = bacc.Bacc("TRN2", target_bir_lowering=False, debug=False,
                   num_devices=N_CORES)

    xT = nc.declare_dram_parameter("xT", [B, D, S], F16, isOutput=False)
    cosT = nc.declare_dram_parameter("cosT", [HD, S], F16, isOutput=False)
    sinT = nc.declare_dram_parameter("sinT", [HD, S], F16, isOutput=False)
    wq = nc.declare_dram_parameter("wq", [D, QH * HD], F16, isOutput=False)
    wk = nc.declare_dram_parameter("wk", [D, HD], F16, isOutput=False)
    wv = nc.declare_dram_parameter("wv", [D, HD], F16, isOutput=False)
    wo = nc.declare_dram_parameter("wo", [QH * HD, D], F16, isOutput=False)
    tri = nc.declare_dram_parameter("tri", [128, 128], F16, isOutput=False)
    onesc = nc.declare_dram_parameter("onesc", [128, 1], F16, isOutput=False)
    onesr = nc.declare_dram_parameter("onesr", [1, 128], F16, isOutput=False)
    yT = nc.declare_dram_parameter("yT", [B, D, S], F16, isOutput=True)

    xT_ap, yT_ap = xT.ap(), yT.ap()

    with tile.TileContext(nc) as tc:
        from contextlib import ExitStack
        with ExitStack() as top:
            const = top.enter_context(tc.tile_pool(name="const", bufs=1))
            stream = top.enter_context(tc.tile_pool(name="stream", bufs=1))

            wq_sb = const.tile([128, CT, QH * HD], F16, name="wq_sb")
            wk_sb = const.tile([128, CT, HD], F16, name="wk_sb")
            wv_sb = const.tile([128, CT, HD], F16, name="wv_sb")
            wo_sb = const.tile([128, QH, D], F16, name="wo_sb")
            cos_sb = const.tile([128, S], F16, name="cos_sb")
            sin_sb = const.tile([128, S], F16, name="sin_sb")
            tri_sb = const.tile([128, 128], F16, name="tri_sb")
            ones_col = const.tile([128, 1], F16, name="ones_col")
            ones_row = const.tile([1, 128], F16, name="ones_row")

            # Constants + wo go through the idle GpSimd (SWDGE) queue,
            # deferred until after the first chunk is emitted so they don't
            # compete with the startup xt stream; the Sync queue carries
            # the latency-critical xt loads.  wq/wk/wv slabs are loaded
            # just-in-time inside the first chunk's ct loop.
            const_loads = []

            def emit_const_loads():
                while const_loads:
                    const_loads.pop(0)()

            const_loads.append(lambda: nc.gpsimd.dma_start(cos_sb[:],
                                                           cosT.ap()))
            const_loads.append(lambda: nc.gpsimd.dma_start(sin_sb[:],
                                                           sinT.ap()))
            const_loads.append(lambda: nc.gpsimd.dma_start(tri_sb[:],
                                                           tri.ap()))
            const_loads.append(lambda: nc.gpsimd.dma_start(ones_col[:],
                                                           onesc.ap()))
            const_loads.append(lambda: nc.gpsimd.dma_start(ones_row[:],
                                                           onesr.ap()))
            for hh in range(QH):
                const_loads.append(
                    lambda h=hh: nc.gpsimd.dma_start(
                        wo_sb[:, h, :], wo.ap()[h * 128:(h + 1) * 128, :]))

            for b in range(B):
                with ExitStack() as bctx:
                    bpool = bctx.enter_context(
                        tc.tile_pool(name=f"b{b}_persist", bufs=1))
                    K_cks = [bpool.tile([128, SC], F16, name=f"K{b}_{s_}")
                             for s_ in range(NSC)]
                    V_cks = [bpool.tile([128, PB, 128], F16, name=f"V{b}_{s_}")
                             for s_ in range(NSC)]
                    Q_cks = [[bpool.tile([128, SC], F16, name=f"Q{b}_{i}_{s_}")
                              for s_ in range(NSC)] for i in range(QH)]

                    # ---------------- QKV projection + RoPE ----------------
                    with ExitStack() as pctx:
                        pps = pctx.enter_context(
                            tc.tile_pool(name=f"b{b}_qkv_ps", bufs=1,
                                         space="PSUM"))
                        sp = pctx.enter_context(
                            tc.tile_pool(name=f"b{b}_qkv_sb", bufs=1))

                        # Per chunk: matmuls + ACT-side PSUM drain; the
                        # rotate/transpose DMA dispatches are deferred one
                        # chunk and emitted on Sync AFTER the next chunk's
                        # xt dispatches, by which time their inputs are
                        # long ready - the Sync queue never blocks.
                        rope_fin = [None] * NSC

                        def emit_rope_finish(sc):
                            cs = slice(sc * SC, (sc + 1) * SC)
                            t_sbs, vsb = rope_fin[sc]
                            rope_dsts = [Q_cks[i][sc] for i in range(QH)]
                            rope_dsts.append(K_cks[sc])
                            rots = []
                            for j in range(4):
                                t_sb = t_sbs[j]
                                rot = sp.tile([128, SC], F16, tag="rot",
                                              bufs=4, name="rot")
                                nc.sync.dma_start(rot[0:64, :],
                                                  t_sb[64:128, :])
                                nc.sync.dma_start(rot[64:128, :],
                                                  t_sb[0:64, :])
                                rots.append(rot)
                            for jj in range(PB):
                                nc.sync.dma_start_transpose(
                                    V_cks[sc][:, jj, :],
                                    vsb[:, jj * 128:(jj + 1) * 128])
                            for j, dst in enumerate(rope_dsts):
                                t_sb, rot = t_sbs[j], rots[j]
                                tmp1 = sp.tile([128, SC], F16, tag="tmp1",
                                               bufs=3, name="tmp1")
                                nc.vector.tensor_mul(tmp1[:], t_sb[:],
                                                     cos_sb[:, cs])
                                tmp2 = sp.tile([128, SC], F16, tag="tmp2",
                                               bufs=3, name="tmp2")
                                nc.vector.tensor_mul(tmp2[:], rot[:],
                                                     sin_sb[:, cs])
                                nc.vector.tensor_add(dst[:], tmp1[:],
                                                     tmp2[:])

                        for sc in range(NSC):
                            cs = slice(sc * SC, (sc + 1) * SC)
                            accs = [pps.tile([128, SC], F32, tag="qkvacc",
                                             bufs=QH + 4, name=f"acc{j}")
                                    for j in range(QH + 2)]
                            for ct in range(CT):
                                if b == 0 and sc == 0:
                                    sl = slice(ct * 128, (ct + 1) * 128)
                                    nc.sync.dma_start(wq_sb[:, ct, :],
                                                      wq.ap()[sl, :])
                                    nc.sync.dma_start(wk_sb[:, ct, :],
                                                      wk.ap()[sl, :])
                                    nc.sync.dma_start(wv_sb[:, ct, :],
                                                      wv.ap()[sl, :])
                                xt = stream.tile([128, SC], F16, tag="x",
                                                 bufs=14, name="xt")
                                nc.sync.dma_start(
                                    xt[:],
                                    xT_ap[b, ct * 128:(ct + 1) * 128, cs])
                                st, sp_ = (ct == 0), (ct == CT - 1)
                                for j in range(QH):
                                    nc.tensor.matmul(
                                        accs[j][:],
                                        wq_sb[:, ct, j * HD:(j + 1) * HD],
                                        xt[:], start=st, stop=sp_)
                                nc.tensor.matmul(accs[QH][:], wk_sb[:, ct, :],
                                                 xt[:], start=st, stop=sp_)
                                nc.tensor.matmul(accs[QH + 1][:],
                                                 wv_sb[:, ct, :],
                                                 xt[:], start=st, stop=sp_)

                            # drain the five PSUM accumulators on ACT
                            t_sbs = []
                            for j in range(4):
                                t_sb = sp.tile([128, SC], F16, tag="tsb",
                                               bufs=8, name="t_sb")
                                nc.scalar.copy(t_sb[:], accs[j][:])
                                t_sbs.append(t_sb)
                            vsb = sp.tile([128, SC], F16, tag="vsb", bufs=2,
                                          name="vsb")
                            nc.scalar.copy(vsb[:], accs[QH + 1][:])
                            rope_fin[sc] = (t_sbs, vsb)

                            if b == 0 and sc == 0:
                                emit_const_loads()
                            if sc > 0:
                                emit_rope_finish(sc - 1)
                        emit_rope_finish(NSC - 1)

                    # ------------- attention + deferred out-projection -------------
                    with ExitStack() as actx:
                        aps = actx.enter_context(
                            tc.tile_pool(name=f"b{b}_attn_ps", bufs=1,
                                         space="PSUM"))
                        asb = actx.enter_context(
                            tc.tile_pool(name=f"b{b}_attn_sb", bufs=1))

                        ohs = {}
                        ycopy_flip = [0]

                        def emit_out_unit(qc, mt):
                            """One out-projection unit: y[:, mt-slice] for
                            chunk qc (3 matmuls + copy + DMA)."""
                            qs = slice(qc * SC, (qc + 1) * SC)
                            y_t = aps.tile([128, SC], F32, tag="y", bufs=2,
                                           name="y_t")
                            for hh in range(QH):
                                nc.tensor.matmul(
                                    y_t[:],
                                    wo_sb[:, hh, mt * 128:(mt + 1) * 128],
                                    ohs[(qc, hh)][:],
                                    start=(hh == 0), stop=(hh == QH - 1))
                            y_sb = asb.tile([128, SC], F16, tag="ysb", bufs=6,
                                            name="y_sb")
                            if ycopy_flip[0] % 2 == 0:
                                nc.scalar.copy(y_sb[:], y_t[:])
                            else:
                                nc.vector.tensor_copy(y_sb[:], y_t[:])
                            ycopy_flip[0] += 1
                            # y writeback via the idle GpSimd SWDGE queue.
                            nc.gpsimd.dma_start(
                                out=yT_ap[b, mt * 128:(mt + 1) * 128, qs],
                                in_=y_sb[:])

                        pending = []

                        for qc in range(NSC):
                            qs = slice(qc * SC, (qc + 1) * SC)
                            # (kt, off): off>0 on causal-diagonal tiles.
                            kts = []
                            for kt in range(4 * qc + PB):
                                off = max(0, (kt - 4 * qc)) * 128 \
                                    if kt >= 4 * qc else 0
                                kts.append((kt, off))
                            n = len(kts)

                            for hh in range(QH):
                                av_t = aps.tile([128, SC], F32, tag="av",
                                                bufs=2, name="av_t")
                                r_t = aps.tile([1, SC], F32, tag="r", bufs=1,
                                               name="r_t")

                                def emit_s_e(i):
                                    kt, off = kts[i]
                                    kb, kj = divmod(kt, PB)
                                    s_t = aps.tile([128, SC], F32, tag="s",
                                                   bufs=3, name="s_t")
                                    nc.tensor.matmul(
                                        s_t[:, off:],
                                        K_cks[kb][:, kj * 128:(kj + 1) * 128],
                                        Q_cks[hh][qc][:, off:],
                                        start=True, stop=True)
                                    e_t = asb.tile([128, SC], F16, tag="e",
                                                   bufs=8, name="e_t")
                                    nc.scalar.activation(
                                        e_t[:, off:], s_t[:, off:], AFT.Exp,
                                        scale=SCALE)
                                    if kt >= 4 * qc:
                                        # diagonal tile: mask the 128-wide
                                        # triangle strip in place.
                                        nc.vector.tensor_mul(
                                            e_t[:, off:off + 128],
                                            e_t[:, off:off + 128],
                                            tri_sb[:])
                                    return e_t

                                # 2-deep score lookahead: the PE queue runs
                                # scores(i+2) while exp(i) finishes on ACT.
                                pipe = [emit_s_e(0)]
                                if n > 1:
                                    pipe.append(emit_s_e(1))
                                for i in range(n):
                                    if i + 2 < n:
                                        pipe.append(emit_s_e(i + 2))
                                    e_t = pipe.pop(0)
                                    kt, off = kts[i]
                                    kb, kj = divmod(kt, PB)
                                    st, sp_ = (i == 0), (i == n - 1)
                                    nc.tensor.matmul(
                                        av_t[:, off:], V_cks[kb][:, kj, :],
                                        e_t[:, off:], start=st, stop=sp_,
                                        skip_group_check=True)
                                    nc.tensor.matmul(
                                        r_t[:, off:], ones_col[:],
                                        e_t[:, off:], start=st, stop=sp_,
                                        skip_group_check=True)

                                # normalization: 1/rowsum broadcast via a
                                # K=1 fp32r matmul; oh = av * invb on DVE.
                                inv_sb = asb.tile([1, SC], F32, tag="inv",
                                                  bufs=2, name="inv_sb")
                                nc.vector.reciprocal_approx_fast(inv_sb[:],
                                                                 r_t[:])
                                inv_r = asb.tile([1, SC], F16, tag="invr",
                                                 bufs=2, name="inv_r")
                                nc.vector.tensor_copy(inv_r[:], inv_sb[:])
                                ib_t = aps.tile([128, SC], F32, tag="av",
                                                bufs=2, name="ib_t")
                                nc.tensor.matmul(
                                    ib_t[:], ones_row[:], inv_r[:],
                                    start=True, stop=True)
                                ib_sb = asb.tile([128, SC], F32, tag="ibsb",
                                                 bufs=2, name="ib_sb")
                                nc.scalar.copy(ib_sb[:], ib_t[:])
                                oh = asb.tile([128, SC], F16, tag="oh",
                                              bufs=6, name="oh")
                                nc.vector.tensor_mul(oh[:], av_t[:],
                                                     ib_sb[:])
                                ohs[(qc, hh)] = oh

                                # drain deferred out-projection as filler
                                for _ in range(FILL_PER_HEAD):
                                    if pending:
                                        pending.pop(0)()

                            # queue this chunk's out-projection; last chunk
                            # drains immediately (batch-boundary filler).
                            pending.extend(
                                (lambda q=qc, m=mt: emit_out_unit(q, m))
                                for mt in range(CT))
                            if qc == NSC - 1:
                                while pending:
                                    pending.pop(0)()

    nc.compile()
    return nc


def make_inputs(x, freqs_cos, freqs_sin, mask, wq, wk, wv, wo):
    f32, f16 = np.float32, np.float16
    x = np.asarray(x, f32)
    xT = np.ascontiguousarray(np.transpose(x, (0, 2, 1)).astype(f16))
    cosT = np.concatenate([freqs_cos, freqs_cos], axis=1).T.astype(f32)
    sinT = np.concatenate([freqs_sin, freqs_sin], axis=1).T.astype(f32)
    sinT[:HD // 2] *= -1.0  # rotate-half sign folded into sin
    cosT = np.ascontiguousarray(cosT.astype(f16))
    sinT = np.ascontiguousarray(sinT.astype(f16))

    # upper-triangle (incl diagonal) ones strip; identical for every
    # causal-diagonal 128-block.
    tri = np.triu(np.ones((128, 128), f32)).astype(f16)

    wqT = np.asarray(wq, f32).T.astype(f16)
    wkT = np.asarray(wk, f32).T.astype(f16)
    wvT = np.asarray(wv, f32).T.astype(f16)
    woT = np.asarray(wo, f32).T.astype(f16)

    in_maps = []
    for h in range(N_CORES):
        qsl = slice(h * QH * HD, (h + 1) * QH * HD)
        ksl = slice(h * HD, (h + 1) * HD)
        in_maps.append({
            "xT": xT,
            "cosT": cosT,
            "sinT": sinT,
            "wq": np.ascontiguousarray(wqT[:, qsl]),
            "wk": np.ascontiguousarray(wkT[:, ksl]),
            "wv": np.ascontiguousarray(wvT[:, ksl]),
            "wo": np.ascontiguousarray(woT[qsl, :]),
            "tri": tri,
            "onesc": np.ones((128, 1), f16),
            "onesr": np.ones((1, 128), f16),
        })
    return in_maps


_CACHE = {}


def kernel(x, freqs_cos, freqs_sin, mask, wq, wk, wv, wo):
    global LAST_EXEC_NS, LAST_RESULTS
    assert tuple(x.shape) == (B, S, D), x.shape

    in_maps = make_inputs(x, freqs_cos, freqs_sin, mask, wq, wk, wv, wo)

    if "v2" not in _CACHE:
        _CACHE["v2"] = build_program()
    nc = _CACHE["v2"]

    kwargs = {}
    if TRACE:
        kwargs = dict(trace=True, trace_cores=[0])
    res = run_bass_kernel_spmd(nc, in_maps, list(range(N_CORES)), **kwargs)
    LAST_EXEC_NS = res.exec_time_ns
    LAST_RESULTS = res

    acc = np.zeros((B, D, S), np.float32)
    for i in range(N_CORES):
        acc += res.results[i]["yT"].astype(np.float32)
    y = np.ascontiguousarray(np.transpose(acc, (0, 2, 1)).astype(np.float32))
    return y


# revision 17
# speedup vs baseline: 1.0260x; 1.0260x over previous
"""GQA attention kernel for Trainium2 (8 NeuronCores, Bass/Tile) — v2.

Problem: B=2, S=2048, D=3072, 24 Q heads / 8 KV heads, HD=128, RoPE,
additive causal mask, softmax, output projection.

Sharding: tensor-parallel over heads. Core h owns KV head h and Q heads
{3h, 3h+1, 3h+2} for both batch elements. Each core produces a partial
y^T = wo_slice^T.T @ attn_out_heads^T of shape (B, D, S) in fp16; the
host sums the 8 partials in fp32 and transposes back.

v2 changes vs v1 (667us baseline):
  - Attention inner loop software-pipelined: scores(kt+1) is emitted
    before AV(kt)/rowsum(kt), so the in-order PE queue never waits on
    the ACT-engine exp.  (v1 lost ~500-1400ns per k-tile here.)
  - Out-projection of chunk qc-1 is deferred and drained between the
    attention heads of chunk qc as PE filler during ACT-paced stretches.
  - Causal diagonal blocks compute only the q >= r*128 live sub-range;
    the mask reduces to a single constant 128x128 upper-triangle tile
    multiplied into a 128-wide strip (no mask DMA traffic at all).
  - RoPE runs fully in fp16 (2x DVE rate), PSUM-drain copies are split
    across ACT and DVE, rotate-half stays a partition-shifted SBUF DMA.
  - V is transposed to [s, d] by the DMA crossbar (dma_start_transpose)
    instead of PE transposes + DVE copies.
  - yT output in fp16 (halves DRAM write traffic), cos/sin loaded once.
"""

import math
import os
import sys

import numpy as np

for _p in ("/opt/trn_rl_repo",):
    if os.path.isdir(_p) and _p not in sys.path:
        sys.path.insert(0, _p)

import concourse.bass as bass  # noqa: E402
import concourse.mybir as mybir  # noqa: E402
import concourse.tile as tile  # noqa: E402
from concourse import bacc  # noqa: E402
from concourse.bass_utils import run_bass_kernel_spmd  # noqa: E402

F32 = mybir.dt.float32
F32R = mybir.dt.float32r
F16 = mybir.dt.float16
AFT = mybir.ActivationFunctionType

N_CORES = 8

TRACE = False
LAST_EXEC_NS = None
LAST_RESULTS = None

B, S, D = 2, 2048, 3072
QH, HD, SC = 3, 128, 512
CT = D // 128          # 24 contraction tiles
KT = S // 128          # 16 key tiles
NSC = S // SC          # 4 token chunks
PB = SC // 128         # 4 key tiles per chunk
SCALE = 1.0 / math.sqrt(HD)

# Filler drain rate: out-projection mt-units emitted after each attention
# head (24 units per chunk / 3 heads).
FILL_PER_HEAD = 8


def build_program():
    nc = bacc.Bacc("TRN2", target_bir_lowering=False, debug=False,
                   num_devices=N_CORES)

    xT = nc.declare_dram_parameter("xT", [B, D, S], F16, isOutput=False)
    cosT = nc.declare_dram_parameter("cosT", [HD, S], F16, isOutput=False)
    sinT = nc.declare_dram_parameter("sinT", [HD, S], F16, isOutput=False)
    wq = nc.declare_dram_parameter("wq", [D, QH * HD], F16, isOutput=False)
    wk = nc.declare_dram_parameter("wk", [D, HD], F16, isOutput=False)
    wv = nc.declare_dram_parameter("wv", [D, HD], F16, isOutput=False)
    wo = nc.declare_dram_parameter("wo", [QH * HD, D], F16, isOutput=False)
    tri = nc.declare_dram_parameter("tri", [128, 128], F16, isOutput=False)
    onesc = nc.declare_dram_parameter("onesc", [128, 1], F16, isOutput=False)
    onesr = nc.declare_dram_parameter("onesr", [1, 128], F16, isOutput=False)
    yT = nc.declare_dram_parameter("yT", [B, D, S], F16, isOutput=True)

    xT_ap, yT_ap = xT.ap(), yT.ap()

    with tile.TileContext(nc) as tc:
        from contextlib import ExitStack
        with ExitStack() as top:
            const = top.enter_context(tc.tile_pool(name="const", bufs=1))
            stream = top.enter_context(tc.tile_pool(name="stream", bufs=1))

            wq_sb = const.tile([128, CT, QH * HD], F16, name="wq_sb")
            wk_sb = const.tile([128, CT, HD], F16, name="wk_sb")
            wv_sb = const.tile([128, CT, HD], F16, name="wv_sb")
            wo_sb = const.tile([128, QH, D], F16, name="wo_sb")
            cos_sb = const.tile([128, S], F16, name="cos_sb")
            sin_sb = const.tile([128, S], F16, name="sin_sb")
            tri_sb = const.tile([128, 128], F16, name="tri_sb")
            ones_col = const.tile([128, 1], F16, name="ones_col")
            ones_row = const.tile([1, 128], F16, name="ones_row")

            # Constants + wo go through the idle GpSimd (SWDGE) queue,
            # deferred until after the first chunk is emitted so they don't
            # compete with the startup xt stream; the Sync queue carries
            # the latency-critical xt loads.  wq/wk/wv slabs are loaded
            # just-in-time inside the first chunk's ct loop.
            const_loads = []

            def emit_const_loads():
                while const_loads:
                    const_loads.pop(0)()

            const_loads.append(lambda: nc.gpsimd.dma_start(cos_sb[:],
                                                           cosT.ap()))
            const_loads.append(lambda: nc.gpsimd.dma_start(sin_sb[:],
                                                           sinT.ap()))
            const_loads.append(lambda: nc.gpsimd.dma_start(tri_sb[:],
                                                           tri.ap()))
            const_loads.append(lambda: nc.gpsimd.dma_start(ones_col[:],
                                                           onesc.ap()))
            const_loads.append(lambda: nc.gpsimd.dma_start(ones_row[:],
                                                           onesr.ap()))
            for hh in range(QH):
                const_loads.append(
                    lambda h=hh: nc.gpsimd.dma_start(
                        wo_sb[:, h, :], wo.ap()[h * 128:(h + 1) * 128, :]))

            for b in range(B):
                with ExitStack() as bctx:
                    bpool = bctx.enter_context(
                        tc.tile_pool(name=f"b{b}_persist", bufs=1))
                    K_cks = [bpool.tile([128, SC], F16, name=f"K{b}_{s_}")
                             for s_ in range(NSC)]
                    V_cks = [bpool.tile([128, PB, 128], F16, name=f"V{b}_{s_}")
                             for s_ in range(NSC)]
                    Q_cks = [[bpool.tile([128, SC], F16, name=f"Q{b}_{i}_{s_}")
                              for s_ in range(NSC)] for i in range(QH)]

                    # ---------------- QKV projection + RoPE ----------------
                    with ExitStack() as pctx:
                        pps = pctx.enter_context(
                            tc.tile_pool(name=f"b{b}_qkv_ps", bufs=1,
                                         space="PSUM"))
                        sp = pctx.enter_context(
                            tc.tile_pool(name=f"b{b}_qkv_sb", bufs=1))

                        # Per chunk: matmuls + ACT-side PSUM drain; the
                        # rotate/transpose DMA dispatches are deferred one
                        # chunk and emitted on Sync AFTER the next chunk's
                        # xt dispatches, by which time their inputs are
                        # long ready - the Sync queue never blocks.
                        rope_fin = [None] * NSC

                        def emit_rope_finish(sc):
                            cs = slice(sc * SC, (sc + 1) * SC)
                            t_sbs, vsb = rope_fin[sc]
                            rope_dsts = [Q_cks[i][sc] for i in range(QH)]
                            rope_dsts.append(K_cks[sc])
                            rots = []
                            for j in range(4):
                                t_sb = t_sbs[j]
                                rot = sp.tile([128, SC], F16, tag="rot",
                                              bufs=4, name="rot")
                                nc.gpsimd.dma_start(out=rot[0:64, :],
                                                    in_=t_sb[64:128, :])
                                nc.gpsimd.dma_start(out=rot[64:128, :],
                                                    in_=t_sb[0:64, :])
                                rots.append(rot)
                            for jj in range(PB):
                                nc.scalar.dma_start_transpose(
                                    V_cks[sc][:, jj, :],
                                    vsb[:, jj * 128:(jj + 1) * 128])
                            for j, dst in enumerate(rope_dsts):
                                t_sb, rot = t_sbs[j], rots[j]
                                tmp1 = sp.tile([128, SC], F16, tag="tmp1",
                                               bufs=3, name="tmp1")
                                nc.vector.tensor_mul(tmp1[:], t_sb[:],
                                                     cos_sb[:, cs])
                                tmp2 = sp.tile([128, SC], F16, tag="tmp2",
                                               bufs=3, name="tmp2")
                                nc.vector.tensor_mul(tmp2[:], rot[:],
                                                     sin_sb[:, cs])
                                nc.vector.tensor_add(dst[:], tmp1[:],
                                                     tmp2[:])

                        for sc in range(NSC):
                            cs = slice(sc * SC, (sc + 1) * SC)
                            accs = [pps.tile([128, SC], F32, tag="qkvacc",
                                             bufs=QH + 4, name=f"acc{j}")
                                    for j in range(QH + 2)]
                            for ct in range(CT):
                                if b == 0 and sc == 0:
                                    sl = slice(ct * 128, (ct + 1) * 128)
                                    nc.sync.dma_start(wq_sb[:, ct, :],
                                                      wq.ap()[sl, :])
                                    nc.sync.dma_start(wk_sb[:, ct, :],
                                                      wk.ap()[sl, :])
                                    nc.sync.dma_start(wv_sb[:, ct, :],
                                                      wv.ap()[sl, :])
                                xt = stream.tile([128, SC], F16, tag="x",
                                                 bufs=26, name="xt")
                                nc.sync.dma_start(
                                    xt[:],
                                    xT_ap[b, ct * 128:(ct + 1) * 128, cs])
                                st, sp_ = (ct == 0), (ct == CT - 1)
                                for j in range(QH):
                                    nc.tensor.matmul(
                                        accs[j][:],
                                        wq_sb[:, ct, j * HD:(j + 1) * HD],
                                        xt[:], start=st, stop=sp_)
                                nc.tensor.matmul(accs[QH][:], wk_sb[:, ct, :],
                                                 xt[:], start=st, stop=sp_)
                                nc.tensor.matmul(accs[QH + 1][:],
                                                 wv_sb[:, ct, :],
                                                 xt[:], start=st, stop=sp_)

                            # drain the five PSUM accumulators on ACT
                            t_sbs = []
                            for j in range(4):
                                t_sb = sp.tile([128, SC], F16, tag="tsb",
                                               bufs=8, name="t_sb")
                                nc.scalar.copy(t_sb[:], accs[j][:])
                                t_sbs.append(t_sb)
                            vsb = sp.tile([128, SC], F16, tag="vsb", bufs=2,
                                          name="vsb")
                            nc.scalar.copy(vsb[:], accs[QH + 1][:])
                            rope_fin[sc] = (t_sbs, vsb)

                            if b == 0 and sc == 0:
                                emit_const_loads()
                            if sc > 0:
                                emit_rope_finish(sc - 1)
                        emit_rope_finish(NSC - 1)

                    # ------------- attention + deferred out-projection -------------
                    with ExitStack() as actx:
                        aps = actx.enter_context(
                            tc.tile_pool(name=f"b{b}_attn_ps", bufs=1,
                                         space="PSUM"))
                        asb = actx.enter_context(
                            tc.tile_pool(name=f"b{b}_attn_sb", bufs=1))

                        ohs = {}
                        ycopy_flip = [0]

                        def emit_out_unit(qc, mt):
                            """One out-projection unit: y[:, mt-slice] for
                            chunk qc (3 matmuls + copy + DMA)."""
                            qs = slice(qc * SC, (qc + 1) * SC)
                            y_t = aps.tile([128, SC], F32, tag="ybi", bufs=2,
                                           name="y_t")
                            for hh in range(QH):
                                nc.tensor.matmul(
                                    y_t[:],
                                    wo_sb[:, hh, mt * 128:(mt + 1) * 128],
                                    ohs[(qc, hh)][:],
                                    start=(hh == 0), stop=(hh == QH - 1))
                            y_sb = asb.tile([128, SC], F16, tag="ysb", bufs=6,
                                            name="y_sb")
                            if ycopy_flip[0] % 2 == 0:
                                nc.scalar.copy(y_sb[:], y_t[:])
                            else:
                                nc.vector.tensor_copy(y_sb[:], y_t[:])
                            ycopy_flip[0] += 1
                            # y writeback via the idle GpSimd SWDGE queue.
                            nc.gpsimd.dma_start(
                                out=yT_ap[b, mt * 128:(mt + 1) * 128, qs],
                                in_=y_sb[:])

                        pending = []

                        for qc in range(NSC):
                            qs = slice(qc * SC, (qc + 1) * SC)
                            # (kt, off): off>0 on causal-diagonal tiles.
                            kts = []
                            for kt in range(4 * qc + PB):
                                off = max(0, (kt - 4 * qc)) * 128 \
                                    if kt >= 4 * qc else 0
                                kts.append((kt, off))
                            n = len(kts)

                            for hh in range(QH):
                                av_t = aps.tile([128, SC], F32, tag="av",
                                                bufs=2, name="av_t")
                                r_t = aps.tile([1, SC], F32, tag="r", bufs=1,
                                               name="r_t")

                                def emit_s_e(i):
                                    kt, off = kts[i]
                                    kb, kj = divmod(kt, PB)
                                    s_t = aps.tile([128, SC], F32, tag="s",
                                                   bufs=3, name="s_t")
                                    nc.tensor.matmul(
                                        s_t[:, off:],
                                        K_cks[kb][:, kj * 128:(kj + 1) * 128],
                                        Q_cks[hh][qc][:, off:],
                                        start=True, stop=True)
                                    e_t = asb.tile([128, SC], F16, tag="e",
                                                   bufs=8, name="e_t")
                                    nc.scalar.activation(
                                        e_t[:, off:], s_t[:, off:], AFT.Exp,
                                        scale=SCALE)
                                    if kt >= 4 * qc:
                                        # diagonal tile: mask the 128-wide
                                        # triangle strip in place.
                                        nc.vector.tensor_mul(
                                            e_t[:, off:off + 128],
                                            e_t[:, off:off + 128],
                                            tri_sb[:])
                                    return e_t

                                # 2-deep score lookahead: the PE queue runs
                                # scores(i+2) while exp(i) finishes on ACT.
                                pipe = [emit_s_e(0)]
                                if n > 1:
                                    pipe.append(emit_s_e(1))
                                for i in range(n):
                                    if i + 2 < n:
                                        pipe.append(emit_s_e(i + 2))
                                    e_t = pipe.pop(0)
                                    kt, off = kts[i]
                                    kb, kj = divmod(kt, PB)
                                    st, sp_ = (i == 0), (i == n - 1)
                                    nc.tensor.matmul(
                                        av_t[:, off:], V_cks[kb][:, kj, :],
                                        e_t[:, off:], start=st, stop=sp_,
                                        skip_group_check=True)
                                    nc.tensor.matmul(
                                        r_t[:, off:], ones_col[:],
                                        e_t[:, off:], start=st, stop=sp_,
                                        skip_group_check=True)

                                # normalization: 1/rowsum broadcast via a
                                # K=1 fp32r matmul; oh = av * invb on DVE.
                                inv_sb = asb.tile([1, SC], F32, tag="inv",
                                                  bufs=2, name="inv_sb")
                                nc.vector.reciprocal_approx_fast(inv_sb[:],
                                                                 r_t[:])
                                inv_r = asb.tile([1, SC], F16, tag="invr",
                                                 bufs=2, name="inv_r")
                                nc.vector.tensor_copy(inv_r[:], inv_sb[:])
                                ib_t = aps.tile([128, SC], F32, tag="ybi",
                                                bufs=2, name="ib_t")
                                nc.tensor.matmul(
                                    ib_t[:], ones_row[:], inv_r[:],
                                    start=True, stop=True)
                                ib_sb = asb.tile([128, SC], F32, tag="ibsb",
                                                 bufs=2, name="ib_sb")
                                nc.vector.tensor_copy(ib_sb[:], ib_t[:])
                                oh = asb.tile([128, SC], F16, tag="oh",
                                              bufs=6, name="oh")
                                nc.vector.tensor_mul(oh[:], av_t[:],
                                                     ib_sb[:])
                                ohs[(qc, hh)] = oh

                                # drain deferred out-projection as filler
                                for _ in range(FILL_PER_HEAD):
                                    if pending:
                                        pending.pop(0)()

                            # queue this chunk's out-projection; last chunk
                            # drains immediately (batch-boundary filler).
                            pending.extend(
                                (lambda q=qc, m=mt: emit_out_unit(q, m))
                                for mt in range(CT))
                            if qc == NSC - 1:
                                while pending:
                                    pending.pop(0)()

    nc.compile()
    return nc


def make_inputs(x, freqs_cos, freqs_sin, mask, wq, wk, wv, wo):
    f32, f16 = np.float32, np.float16
    x = np.asarray(x, f32)
    xT = np.ascontiguousarray(np.transpose(x, (0, 2, 1)).astype(f16))
    cosT = np.concatenate([freqs_cos, freqs_cos], axis=1).T.astype(f32)
    sinT = np.concatenate([freqs_sin, freqs_sin], axis=1).T.astype(f32)
    sinT[:HD // 2] *= -1.0  # rotate-half sign folded into sin
    cosT = np.ascontiguousarray(cosT.astype(f16))
    sinT = np.ascontiguousarray(sinT.astype(f16))

    # upper-triangle (incl diagonal) ones strip; identical for every
    # causal-diagonal 128-block.
    tri = np.triu(np.ones((128, 128), f32)).astype(f16)

    wqT = np.asarray(wq, f32).T.astype(f16)
    wkT = np.asarray(wk, f32).T.astype(f16)
    wvT = np.asarray(wv, f32).T.astype(f16)
    woT = np.asarray(wo, f32).T.astype(f16)

    in_maps = []
    for h in range(N_CORES):
        qsl = slice(h * QH * HD, (h + 1) * QH * HD)
        ksl = slice(h * HD, (h + 1) * HD)
        in_maps.append({
            "xT": xT,
            "cosT": cosT,
            "sinT": sinT,
            "wq": np.ascontiguousarray(wqT[:, qsl]),
            "wk": np.ascontiguousarray(wkT[:, ksl]),
            "wv": np.ascontiguousarray(wvT[:, ksl]),
            "wo": np.ascontiguousarray(woT[qsl, :]),
            "tri": tri,
            "onesc": np.ones((128, 1), f16),
            "onesr": np.ones((1, 128), f16),
        })
    return in_maps


_CACHE = {}


def kernel(x, freqs_cos, freqs_sin, mask, wq, wk, wv, wo):
    global LAST_EXEC_NS, LAST_RESULTS
    assert tuple(x.shape) == (B, S, D), x.shape

    in_maps = make_inputs(x, freqs_cos, freqs_sin, mask, wq, wk, wv, wo)

    if "v2" not in _CACHE:
        _CACHE["v2"] = build_program()
    nc = _CACHE["v2"]

    kwargs = {}
    if TRACE:
        kwargs = dict(trace=True, trace_cores=[0])
    res = run_bass_kernel_spmd(nc, in_maps, list(range(N_CORES)), **kwargs)
    LAST_EXEC_NS = res.exec_time_ns
    LAST_RESULTS = res

    acc = np.zeros((B, D, S), np.float32)
    for i in range(N_CORES):
        acc += res.results[i]["yT"].astype(np.float32)
    y = np.ascontiguousarray(np.transpose(acc, (0, 2, 1)).astype(np.float32))
    return y


# revision 18
# speedup vs baseline: 1.0404x; 1.0140x over previous
"""GQA attention kernel for Trainium2 (8 NeuronCores, Bass/Tile) — v2.

Problem: B=2, S=2048, D=3072, 24 Q heads / 8 KV heads, HD=128, RoPE,
additive causal mask, softmax, output projection.

Sharding: tensor-parallel over heads. Core h owns KV head h and Q heads
{3h, 3h+1, 3h+2} for both batch elements. Each core produces a partial
y^T = wo_slice^T.T @ attn_out_heads^T of shape (B, D, S) in fp16; the
host sums the 8 partials in fp32 and transposes back.

v2 changes vs v1 (667us baseline):
  - Attention inner loop software-pipelined: scores(kt+1) is emitted
    before AV(kt)/rowsum(kt), so the in-order PE queue never waits on
    the ACT-engine exp.  (v1 lost ~500-1400ns per k-tile here.)
  - Out-projection of chunk qc-1 is deferred and drained between the
    attention heads of chunk qc as PE filler during ACT-paced stretches.
  - Causal diagonal blocks compute only the q >= r*128 live sub-range;
    the mask reduces to a single constant 128x128 upper-triangle tile
    multiplied into a 128-wide strip (no mask DMA traffic at all).
  - RoPE runs fully in fp16 (2x DVE rate), PSUM-drain copies are split
    across ACT and DVE, rotate-half stays a partition-shifted SBUF DMA.
  - V is transposed to [s, d] by the DMA crossbar (dma_start_transpose)
    instead of PE transposes + DVE copies.
  - yT output in fp16 (halves DRAM write traffic), cos/sin loaded once.
"""

import math
import os
import sys

import numpy as np

for _p in ("/opt/trn_rl_repo",):
    if os.path.isdir(_p) and _p not in sys.path:
        sys.path.insert(0, _p)

import concourse.bass as bass  # noqa: E402
import concourse.mybir as mybir  # noqa: E402
import concourse.tile as tile  # noqa: E402
from concourse import bacc  # noqa: E402
from concourse.bass_utils import run_bass_kernel_spmd  # noqa: E402

F32 = mybir.dt.float32
F32R = mybir.dt.float32r
F16 = mybir.dt.float16
AFT = mybir.ActivationFunctionType

N_CORES = 8

TRACE = False
LAST_EXEC_NS = None
LAST_RESULTS = None

B, S, D = 2, 2048, 3072
QH, HD, SC = 3, 128, 512
CT = D // 128          # 24 contraction tiles
KT = S // 128          # 16 key tiles
NSC = S // SC          # 4 token chunks
PB = SC // 128         # 4 key tiles per chunk
SCALE = 1.0 / math.sqrt(HD)

# Filler drain rate: out-projection mt-units emitted after each attention
# head (24 units per chunk / 3 heads).
FILL_PER_HEAD = 8


def build_program():
    nc = bacc.Bacc("TRN2", target_bir_lowering=False, debug=False,
                   num_devices=N_CORES)

    xT = nc.declare_dram_parameter("xT", [B, D, S], F16, isOutput=False)
    cosT = nc.declare_dram_parameter("cosT", [HD, S], F16, isOutput=False)
    sinT = nc.declare_dram_parameter("sinT", [HD, S], F16, isOutput=False)
    wq = nc.declare_dram_parameter("wq", [D, QH * HD], F16, isOutput=False)
    wk = nc.declare_dram_parameter("wk", [D, HD], F16, isOutput=False)
    wv = nc.declare_dram_parameter("wv", [D, HD], F16, isOutput=False)
    wo = nc.declare_dram_parameter("wo", [QH * HD, D], F16, isOutput=False)
    tri = nc.declare_dram_parameter("tri", [128, 128], F16, isOutput=False)
    onesc = nc.declare_dram_parameter("onesc", [128, 1], F16, isOutput=False)
    onesr = nc.declare_dram_parameter("onesr", [1, 128], F16, isOutput=False)
    yT = nc.declare_dram_parameter("yT", [B, D, S], F16, isOutput=True)

    xT_ap, yT_ap = xT.ap(), yT.ap()

    with tile.TileContext(nc) as tc:
        from contextlib import ExitStack
        with ExitStack() as top:
            const = top.enter_context(tc.tile_pool(name="const", bufs=1))
            stream = top.enter_context(tc.tile_pool(name="stream", bufs=1))

            wq_sb = const.tile([128, CT, QH * HD], F16, name="wq_sb")
            wk_sb = const.tile([128, CT, HD], F16, name="wk_sb")
            wv_sb = const.tile([128, CT, HD], F16, name="wv_sb")
            wo_sb = const.tile([128, QH, D], F16, name="wo_sb")
            cos_sb = const.tile([128, S], F16, name="cos_sb")
            sin_sb = const.tile([128, S], F16, name="sin_sb")
            tri_sb = const.tile([128, 128], F16, name="tri_sb")
            ones_col = const.tile([128, 1], F16, name="ones_col")
            ones_row = const.tile([1, 128], F16, name="ones_row")

            # Constants + wo go through the idle GpSimd (SWDGE) queue,
            # deferred until after the first chunk is emitted so they don't
            # compete with the startup xt stream; the Sync queue carries
            # the latency-critical xt loads.  wq/wk/wv slabs are loaded
            # just-in-time inside the first chunk's ct loop.
            const_loads = []

            def emit_const_loads():
                while const_loads:
                    const_loads.pop(0)()

            const_loads.append(lambda: nc.gpsimd.dma_start(cos_sb[:],
                                                           cosT.ap()))
            const_loads.append(lambda: nc.gpsimd.dma_start(sin_sb[:],
                                                           sinT.ap()))
            const_loads.append(lambda: nc.gpsimd.dma_start(tri_sb[:],
                                                           tri.ap()))
            const_loads.append(lambda: nc.gpsimd.dma_start(ones_col[:],
                                                           onesc.ap()))
            const_loads.append(lambda: nc.gpsimd.dma_start(ones_row[:],
                                                           onesr.ap()))
            for hh in range(QH):
                const_loads.append(
                    lambda h=hh: nc.gpsimd.dma_start(
                        wo_sb[:, h, :], wo.ap()[h * 128:(h + 1) * 128, :]))

            for b in range(B):
                with ExitStack() as bctx:
                    bpool = bctx.enter_context(
                        tc.tile_pool(name=f"b{b}_persist", bufs=1))
                    K_cks = [bpool.tile([128, SC], F16, name=f"K{b}_{s_}")
                             for s_ in range(NSC)]
                    V_cks = [bpool.tile([128, PB, 128], F16, name=f"V{b}_{s_}")
                             for s_ in range(NSC)]
                    Q_cks = [[bpool.tile([128, SC], F16, name=f"Q{b}_{i}_{s_}")
                              for s_ in range(NSC)] for i in range(QH)]

                    # ---------------- QKV projection + RoPE ----------------
                    with ExitStack() as pctx:
                        pps = pctx.enter_context(
                            tc.tile_pool(name=f"b{b}_qkv_ps", bufs=1,
                                         space="PSUM"))
                        sp = pctx.enter_context(
                            tc.tile_pool(name=f"b{b}_qkv_sb", bufs=1))

                        # Per chunk: matmuls + ACT-side PSUM drain; the
                        # rotate/transpose DMA dispatches are deferred one
                        # chunk and emitted on Sync AFTER the next chunk's
                        # xt dispatches, by which time their inputs are
                        # long ready - the Sync queue never blocks.
                        rope_fin = [None] * NSC

                        def emit_rope_finish(sc):
                            cs = slice(sc * SC, (sc + 1) * SC)
                            t_sbs, vsb = rope_fin[sc]
                            rope_dsts = [Q_cks[i][sc] for i in range(QH)]
                            rope_dsts.append(K_cks[sc])
                            rots = []
                            for j in range(4):
                                t_sb = t_sbs[j]
                                rot = sp.tile([128, SC], F16, tag="rot",
                                              bufs=4, name="rot")
                                nc.gpsimd.dma_start(out=rot[0:64, :],
                                                    in_=t_sb[64:128, :])
                                nc.gpsimd.dma_start(out=rot[64:128, :],
                                                    in_=t_sb[0:64, :])
                                rots.append(rot)
                            for jj in range(PB):
                                nc.sync.dma_start_transpose(
                                    V_cks[sc][:, jj, :],
                                    vsb[:, jj * 128:(jj + 1) * 128])
                            for j, dst in enumerate(rope_dsts):
                                t_sb, rot = t_sbs[j], rots[j]
                                tmp1 = sp.tile([128, SC], F16, tag="tmp1",
                                               bufs=3, name="tmp1")
                                nc.vector.tensor_mul(tmp1[:], t_sb[:],
                                                     cos_sb[:, cs])
                                tmp2 = sp.tile([128, SC], F16, tag="tmp2",
                                               bufs=3, name="tmp2")
                                nc.vector.tensor_mul(tmp2[:], rot[:],
                                                     sin_sb[:, cs])
                                nc.vector.tensor_add(dst[:], tmp1[:],
                                                     tmp2[:])

                        for sc in range(NSC):
                            cs = slice(sc * SC, (sc + 1) * SC)
                            accs = [pps.tile([128, SC], F32, tag="qkvacc",
                                             bufs=QH + 5, name=f"acc{j}")
                                    for j in range(QH + 2)]
                            for ct in range(CT):
                                if b == 0 and sc == 0:
                                    sl = slice(ct * 128, (ct + 1) * 128)
                                    nc.sync.dma_start(wq_sb[:, ct, :],
                                                      wq.ap()[sl, :])
                                    nc.sync.dma_start(wk_sb[:, ct, :],
                                                      wk.ap()[sl, :])
                                    nc.sync.dma_start(wv_sb[:, ct, :],
                                                      wv.ap()[sl, :])
                                xt = stream.tile([128, SC], F16, tag="x",
                                                 bufs=26, name="xt")
                                nc.sync.dma_start(
                                    xt[:],
                                    xT_ap[b, ct * 128:(ct + 1) * 128, cs])
                                st, sp_ = (ct == 0), (ct == CT - 1)
                                for j in range(QH):
                                    nc.tensor.matmul(
                                        accs[j][:],
                                        wq_sb[:, ct, j * HD:(j + 1) * HD],
                                        xt[:], start=st, stop=sp_)
                                nc.tensor.matmul(accs[QH][:], wk_sb[:, ct, :],
                                                 xt[:], start=st, stop=sp_)
                                nc.tensor.matmul(accs[QH + 1][:],
                                                 wv_sb[:, ct, :],
                                                 xt[:], start=st, stop=sp_)

                            # drain the five PSUM accumulators on ACT
                            t_sbs = []
                            for j in range(4):
                                t_sb = sp.tile([128, SC], F16, tag="tsb",
                                               bufs=8, name="t_sb")
                                nc.scalar.copy(t_sb[:], accs[j][:])
                                t_sbs.append(t_sb)
                            vsb = sp.tile([128, SC], F16, tag="vsb", bufs=2,
                                          name="vsb")
                            nc.scalar.copy(vsb[:], accs[QH + 1][:])
                            rope_fin[sc] = (t_sbs, vsb)

                            if b == 0 and sc == 0:
                                emit_const_loads()
                            if sc > 0:
                                emit_rope_finish(sc - 1)
                        emit_rope_finish(NSC - 1)

                    # ------------- attention + deferred out-projection -------------
                    with ExitStack() as actx:
                        aps = actx.enter_context(
                            tc.tile_pool(name=f"b{b}_attn_ps", bufs=1,
                                         space="PSUM"))
                        asb = actx.enter_context(
                            tc.tile_pool(name=f"b{b}_attn_sb", bufs=1))

                        ohs = {}
                        ycopy_flip = [0]

                        def emit_out_unit(qc, mt):
                            """One out-projection unit: y[:, mt-slice] for
                            chunk qc (3 matmuls + copy + DMA)."""
                            qs = slice(qc * SC, (qc + 1) * SC)
                            y_t = aps.tile([128, SC], F32, tag="ybi", bufs=2,
                                           name="y_t")
                            for hh in range(QH):
                                nc.tensor.matmul(
                                    y_t[:],
                                    wo_sb[:, hh, mt * 128:(mt + 1) * 128],
                                    ohs[(qc, hh)][:],
                                    start=(hh == 0), stop=(hh == QH - 1))
                            y_sb = asb.tile([128, SC], F16, tag="ysb", bufs=6,
                                            name="y_sb")
                            if ycopy_flip[0] % 2 == 0:
                                nc.scalar.copy(y_sb[:], y_t[:])
                            else:
                                nc.vector.tensor_copy(y_sb[:], y_t[:])
                            ycopy_flip[0] += 1
                            # y writeback via the idle GpSimd SWDGE queue.
                            nc.gpsimd.dma_start(
                                out=yT_ap[b, mt * 128:(mt + 1) * 128, qs],
                                in_=y_sb[:])

                        pending = []

                        for qc in range(NSC):
                            qs = slice(qc * SC, (qc + 1) * SC)
                            # (kt, off): off>0 on causal-diagonal tiles.
                            kts = []
                            for kt in range(4 * qc + PB):
                                off = max(0, (kt - 4 * qc)) * 128 \
                                    if kt >= 4 * qc else 0
                                kts.append((kt, off))
                            n = len(kts)

                            for hh in range(QH):
                                av_t = aps.tile([128, SC], F32, tag="av",
                                                bufs=2, name="av_t")
                                r_t = aps.tile([1, SC], F32, tag="r", bufs=1,
                                               name="r_t")

                                def emit_s_e(i):
                                    kt, off = kts[i]
                                    kb, kj = divmod(kt, PB)
                                    s_t = aps.tile([128, SC], F32, tag="s",
                                                   bufs=3, name="s_t")
                                    nc.tensor.matmul(
                                        s_t[:, off:],
                                        K_cks[kb][:, kj * 128:(kj + 1) * 128],
                                        Q_cks[hh][qc][:, off:],
                                        start=True, stop=True)
                                    e_t = asb.tile([128, SC], F16, tag="e",
                                                   bufs=8, name="e_t")
                                    nc.scalar.activation(
                                        e_t[:, off:], s_t[:, off:], AFT.Exp,
                                        scale=SCALE)
                                    if kt >= 4 * qc:
                                        # diagonal tile: mask the 128-wide
                                        # triangle strip in place.
                                        nc.vector.tensor_mul(
                                            e_t[:, off:off + 128],
                                            e_t[:, off:off + 128],
                                            tri_sb[:])
                                    return e_t

                                # 2-deep score lookahead: the PE queue runs
                                # scores(i+2) while exp(i) finishes on ACT.
                                pipe = [emit_s_e(0)]
                                if n > 1:
                                    pipe.append(emit_s_e(1))
                                for i in range(n):
                                    if i + 2 < n:
                                        pipe.append(emit_s_e(i + 2))
                                    e_t = pipe.pop(0)
                                    kt, off = kts[i]
                                    kb, kj = divmod(kt, PB)
                                    st, sp_ = (i == 0), (i == n - 1)
                                    nc.tensor.matmul(
                                        av_t[:, off:], V_cks[kb][:, kj, :],
                                        e_t[:, off:], start=st, stop=sp_,
                                        skip_group_check=True)
                                    nc.tensor.matmul(
                                        r_t[:, off:], ones_col[:],
                                        e_t[:, off:], start=st, stop=sp_,
                                        skip_group_check=True)

                                # normalization: 1/rowsum broadcast via a
                                # K=1 fp32r matmul; oh = av * invb on DVE.
                                inv_sb = asb.tile([1, SC], F32, tag="inv",
                                                  bufs=2, name="inv_sb")
                                nc.vector.reciprocal_approx_fast(inv_sb[:],
                                                                 r_t[:])
                                inv_r = asb.tile([1, SC], F16, tag="invr",
                                                 bufs=2, name="inv_r")
                                nc.vector.tensor_copy(inv_r[:], inv_sb[:])
                                ib_t = aps.tile([128, SC], F32, tag="ybi",
                                                bufs=2, name="ib_t")
                                nc.tensor.matmul(
                                    ib_t[:], ones_row[:], inv_r[:],
                                    start=True, stop=True)
                                ib_sb = asb.tile([128, SC], F32, tag="ibsb",
                                                 bufs=2, name="ib_sb")
                                nc.vector.tensor_copy(ib_sb[:], ib_t[:])
                                oh = asb.tile([128, SC], F16, tag="oh",
                                              bufs=6, name="oh")
                                nc.vector.tensor_mul(oh[:], av_t[:],
                                                     ib_sb[:])
                                ohs[(qc, hh)] = oh

                                # drain deferred out-projection as filler
                                for _ in range(FILL_PER_HEAD):
                                    if pending:
                                        pending.pop(0)()

                            # queue this chunk's out-projection; last chunk
                            # drains immediately (batch-boundary filler).
                            pending.extend(
                                (lambda q=qc, m=mt: emit_out_unit(q, m))
                                for mt in range(CT))
                            if qc == NSC - 1:
                                while pending:
                                    pending.pop(0)()

    nc.compile()
    return nc


def make_inputs(x, freqs_cos, freqs_sin, mask, wq, wk, wv, wo):
    f32, f16 = np.float32, np.float16
    x = np.asarray(x, f32)
    xT = np.ascontiguousarray(np.transpose(x, (0, 2, 1)).astype(f16))
    cosT = np.concatenate([freqs_cos, freqs_cos], axis=1).T.astype(f32)
    sinT = np.concatenate([freqs_sin, freqs_sin], axis=1).T.astype(f32)
    sinT[:HD // 2] *= -1.0  # rotate-half sign folded into sin
    cosT = np.ascontiguousarray(cosT.astype(f16))
    sinT = np.ascontiguousarray(sinT.astype(f16))

    # upper-triangle (incl diagonal) ones strip; identical for every
    # causal-diagonal 128-block.
    tri = np.triu(np.ones((128, 128), f32)).astype(f16)

    wqT = np.asarray(wq, f32).T.astype(f16)
    wkT = np.asarray(wk, f32).T.astype(f16)
    wvT = np.asarray(wv, f32).T.astype(f16)
    woT = np.asarray(wo, f32).T.astype(f16)

    in_maps = []
    for h in range(N_CORES):
        qsl = slice(h * QH * HD, (h + 1) * QH * HD)
        ksl = slice(h * HD, (h + 1) * HD)
        in_maps.append({
            "xT": xT,
            "cosT": cosT,
            "sinT": sinT,
            "wq": np.ascontiguousarray(wqT[:, qsl]),
            "wk": np.ascontiguousarray(wkT[:, ksl]),
            "wv": np.ascontiguousarray(wvT[:, ksl]),
            "wo": np.ascontiguousarray(woT[qsl, :]),
            "tri": tri,
            "onesc": np.ones((128, 1), f16),
            "onesr": np.ones((1, 128), f16),
        })
    return in_maps


_CACHE = {}


def kernel(x, freqs_cos, freqs_sin, mask, wq, wk, wv, wo):
    global LAST_EXEC_NS, LAST_RESULTS
    assert tuple(x.shape) == (B, S, D), x.shape

    in_maps = make_inputs(x, freqs_cos, freqs_sin, mask, wq, wk, wv, wo)

    if "v2" not in _CACHE:
        _CACHE["v2"] = build_program()
    nc = _CACHE["v2"]

    kwargs = {}
    if TRACE:
        kwargs = dict(trace=True, trace_cores=[0])
    res = run_bass_kernel_spmd(nc, in_maps, list(range(N_CORES)), **kwargs)
    LAST_EXEC_NS = res.exec_time_ns
    LAST_RESULTS = res

    acc = np.zeros((B, D, S), np.float32)
    for i in range(N_CORES):
        acc += res.results[i]["yT"].astype(np.float32)
    y = np.ascontiguousarray(np.transpose(acc, (0, 2, 1)).astype(np.float32))
    return y
